# revision 34
# baseline (speedup 1.0000x reference)
"""Expert-parallel batched-expert FFN kernel for Trainium2 (8 NeuronCores).

Reference computation (per expert e):
    y = relu(x[e] @ fc1_w[e] + fc1_b[e]) @ fc2_w[e] + fc2_b[e]

Sharding: E=8 experts, one expert per core (expert parallel, no collectives).

Per-core algorithm (T=2048 tokens, D=1024, H=4096), token-group-outer:
  - All large operands are repacked host-side (with the fp32->fp16 cast)
    into the exact SBUF slab layouts the kernel consumes, so every device
    DMA is one ~1MB transfer with >=1KB-contiguous per-partition rows:
      xt  [c4*128+p][k][t]  (x transposed to [D,T], chunked by 512 tokens)
      w1  [b*128+p][k][h]   (FC1 lhsT tiles, per 512-wide h block)
      w2  [b*128+p][hk][d]  (FC2 rhs tiles, per block)
  - Both weight matrices are SBUF-resident (16MB fp16 = 128KB/partition);
    they stream in once behind the ramp-critical slabs (x c0 + w1(0)).
  - Outer loop over 4 groups of 512 tokens.  Per group:
      FC1: for each of 32 h-tiles, accumulate 8 k-tile matmuls in PSUM,
           relu (+b1, fused) drains to a [128, 512] yT tile (scalar eng).
      FC2: for each (ti, dc) output tile, accumulate ALL 32 h-tiles in a
           single PSUM pass, then one DVE add (+b2 broadcast) drains to
           SBUF and the 256KB store issues immediately.
    So there is no cross-block SBUF accumulator, x chunks 1-3 stay out of
    the DMA ramp, and output stores spread across the whole run instead
    of flushing 8MB at the end.
  - x chunks use a 2-slot SBUF window; the slot-reuse WAR dependency
    auto-delays chunk c+2's DMA until group c's FC1 finished.
  - Matmul operands are fp16 (m10): inputs round to ~2^-11 relative; all
    accumulation is fp32 in PSUM.  Measured end-to-end L2 relative error
    vs the fp32 reference is ~4e-4.
  - Dependency-free REAL matmuls (not transposes, which don't count as
    PE-busy for the HAM clock gate) at t=0 bring the PE clock to 8/8
    during the DMA-bound lead-in so FC1 starts at full rate.
"""

from contextlib import ExitStack

import numpy as np

import concourse.bass as bass
import concourse.bacc as bacc
import concourse.mybir as mybir
import concourse.tile as tile
from concourse.bass_utils import run_bass_kernel_spmd

E, T, D, H = 8, 2048, 1024, 4096
NCORES = 8
HB = 512           # h per weight block
FP = mybir.dt.float32
FP16 = mybir.dt.float16
RELU = mybir.ActivationFunctionType.Relu

N_BLK = H // HB                # 8   weight blocks
N_HI = HB // 128               # 4   h-tiles per block
N_HK = H // 128                # 32  h-tiles total
N_KI = D // 128                # 8   k-tiles for FC1
N_DC = D // 512                # 2   512-col chunks of D
N_C4 = T // 512                # 4   512-token groups
N_TG = 4                       # ti tiles per token group
N_JUNK = 60                    # HAM warm-up matmuls at t=0


def _emit_kernel(tc, out, xt, w1, b1, w2, b2):
    nc = tc.nc
    with ExitStack() as ctx:
        singles = ctx.enter_context(tc.tile_pool(name="singles", bufs=1))
        xt_pool = ctx.enter_context(tc.tile_pool(name="xt", bufs=1))
        yt_pool = ctx.enter_context(tc.tile_pool(name="yt", bufs=1))
        st_pool = ctx.enter_context(tc.tile_pool(name="st", bufs=4))
        w1_pool = ctx.enter_context(tc.tile_pool(name="w1", bufs=1))
        w2_pool = ctx.enter_context(tc.tile_pool(name="w2", bufs=1))
        psum = ctx.enter_context(tc.tile_pool(name="psum", bufs=4, space="PSUM"))

        # ---- ramp-critical DMA order: x c0 + w1 blocks interleaved over
        # BOTH rings in first-use order; the w2 stream (not needed until
        # FC2(c0) at ~70us) is gated behind a junk-matmul landmark via a
        # dummy-tile WAR dependency, so the first ~13us of chip-wide HBM
        # bandwidth carries only ramp-critical bytes. ----
        # sync ring:   x c0 | w1(4..7) | [gate] w2(4..7) | b2b | x c1
        # scalar ring: b1t | w1(0..3)  | [gate] w2(0..3) | x c2 | x c3
        xTc = [None] * N_C4

        def emit_xload(c4, eng):
            xTc[c4] = xt_pool.tile([128, N_KI, 512], FP16, tag=f"xt{c4 % 2}",
                                   name=f"xT{c4}")
            eng.dma_start(out=xTc[c4], in_=xt[c4 * 128:(c4 + 1) * 128, :, :])

        emit_xload(0, nc.sync)

        b1t = singles.tile([128, N_HK], FP)
        nc.scalar.dma_start(out=b1t, in_=b1)

        w1b = []
        for b in range(N_BLK):
            wb = w1_pool.tile([128, N_KI, HB], FP16, tag=f"w1b{b}",
                              name=f"w1b{b}")
            eng = nc.scalar if b < 4 else nc.sync
            eng.dma_start(out=wb, in_=w1[b * 128:(b + 1) * 128, :, :])
            w1b.append(wb)

        wtile = singles.tile([128, 128], FP16)
        nc.vector.memset(wtile, 0.0)

        # dummy first-generation tiles in every w2 ring slot; junk matmuls
        # near the end of the warm-up train consume them, so the real w2
        # loads (slot reuse = WAR) can't start before ~junk-end.
        w2gate = []
        for b in range(N_BLK):
            dm = w2_pool.tile([128, N_HI, D], FP16, tag=f"w2b{b}",
                              name=f"w2dm{b}")
            nc.vector.memset(dm[:, 0, 0:128], 0.0)
            w2gate.append(dm)

        # ---- HAM warm-up: dependency-free real matmuls on a zero tile
        # bring the PE clock gate to 8/8 during the DMA-bound lead-in.
        for j in range(N_JUNK):
            pt = psum.tile([128, 128], FP, tag="psA", name=f"wu{j}")
            rhs = wtile
            if N_JUNK - 12 <= j < N_JUNK - 4:
                rhs = w2gate[j - (N_JUNK - 12)][:, 0, 0:128]
            nc.tensor.matmul(pt, lhsT=wtile, rhs=rhs, start=True, stop=True)

        w2b = []
        for b in range(N_BLK):
            wb = w2_pool.tile([128, N_HI, D], FP16, tag=f"w2b{b}",
                              name=f"w2b{b}")
            eng = nc.scalar if b < 4 else nc.sync
            eng.dma_start(out=wb, in_=w2[b * 128:(b + 1) * 128, :, :])
            w2b.append(wb)

        b2b = singles.tile([128, D], FP)
        nc.sync.dma_start(out=b2b, in_=b2)

        emit_xload(1, nc.sync)
        emit_xload(2, nc.scalar)   # WAR on slot 0 delays this past FC1(c0)
        emit_xload(3, nc.scalar)   # WAR on slot 1 delays this past FC1(c1)

        yT = [yt_pool.tile([128, 512], FP16, tag=f"yt{hk}", name=f"yT{hk}")
              for hk in range(N_HK)]

        for c4 in range(N_C4):
            # ---- FC1: yT[hk] = relu(w1.T @ x[c4-chunk] + b1) ----
            for b in range(N_BLK):
                pts = [psum.tile([128, 512], FP, tag="psA",
                                 name=f"psfc1_{c4}_{b}_{hi}")
                       for hi in range(N_HI)]
                for hi in range(N_HI):
                    hk = b * N_HI + hi
                    for ki in range(N_KI):
                        nc.tensor.matmul(
                            pts[hi],
                            lhsT=w1b[b][:, ki, hi * 128:(hi + 1) * 128],
                            rhs=xTc[c4][:, ki, :],
                            start=(ki == 0), stop=(ki == N_KI - 1))
                    nc.scalar.activation(
                        out=yT[hk], in_=pts[hi],
                        func=RELU, bias=b1t[:, hk:hk + 1], scale=1.0)

            # ---- FC2: one full-H PSUM pass per (ti, dc) output tile ----
            for ti in range(N_TG):
                gti = c4 * N_TG + ti
                for dc in range(N_DC):
                    pt = psum.tile([128, 512], FP, tag="psB",
                                   name=f"psfc2_{c4}_{ti}_{dc}")
                    for hk in range(N_HK):
                        nc.tensor.matmul(
                            pt,
                            lhsT=yT[hk][:, ti * 128:(ti + 1) * 128],
                            rhs=w2b[hk // N_HI][:, hk % N_HI,
                                                dc * 512:(dc + 1) * 512],
                            start=(hk == 0), stop=(hk == N_HK - 1))
                    st = st_pool.tile([128, 512], FP, tag="st",
                                      name=f"st{gti}_{dc}")
                    nc.vector.tensor_add(
                        st, pt, b2b[:, dc * 512:(dc + 1) * 512])
                    eng = nc.sync if (gti + dc) % 2 == 0 else nc.scalar
                    eng.dma_start(
                        out=out[gti * 128:(gti + 1) * 128,
                                dc * 512:(dc + 1) * 512],
                        in_=st)


def build_module():
    nc = bacc.Bacc("TRN2", target_bir_lowering=False, debug=False)
    xt = nc.dram_tensor("xt", [N_C4 * 128, N_KI, 512], FP16,
                        kind="ExternalInput").ap()
    w1 = nc.dram_tensor("fc1_w", [N_BLK * 128, N_KI, HB], FP16,
                        kind="ExternalInput").ap()
    b1 = nc.dram_tensor("fc1_b", [128, H // 128], FP,
                        kind="ExternalInput").ap()
    w2 = nc.dram_tensor("fc2_w", [N_BLK * 128, N_HI, D], FP16,
                        kind="ExternalInput").ap()
    b2 = nc.dram_tensor("fc2_b", [128, D], FP, kind="ExternalInput").ap()
    out = nc.dram_tensor("out", [T, D], FP, kind="ExternalOutput").ap()
    with tile.TileContext(nc) as tc:
        _emit_kernel(tc, out, xt, w1, b1, w2, b2)
    nc.compile()
    return nc


_CACHED = None


def kernel(x, fc1_w, fc1_b, fc2_w, fc2_b, _trace=False, _trace_cores=None):
    global _CACHED
    if _CACHED is None:
        _CACHED = build_module()
    nc = _CACHED

    # host-side staging: fp16 cast + repack into the kernel's slab layouts
    x16 = np.asarray(x, dtype=np.float32).astype(np.float16)
    w116 = np.asarray(fc1_w, dtype=np.float32).astype(np.float16)
    w216 = np.asarray(fc2_w, dtype=np.float32).astype(np.float16)
    # x [E,T,D] -> xT [E,D,T] -> [E, k, p, c4, t] -> [E, c4, p, k, t]
    xq = np.ascontiguousarray(
        x16.transpose(0, 2, 1).reshape(E, N_KI, 128, N_C4, 512)
           .transpose(0, 3, 2, 1, 4)).reshape(E, N_C4 * 128, N_KI, 512)
    # w1 [E,D,H] -> [E, k, p, b, h] -> [E, b, p, k, h]
    w1q = np.ascontiguousarray(
        w116.reshape(E, N_KI, 128, N_BLK, HB).transpose(0, 3, 2, 1, 4)
    ).reshape(E, N_BLK * 128, N_KI, HB)
    # w2 [E,H,D] -> [E, b, hk, p, d] -> [E, b, p, hk, d]
    w2q = np.ascontiguousarray(
        w216.reshape(E, N_BLK, N_HI, 128, D).transpose(0, 1, 3, 2, 4)
    ).reshape(E, N_BLK * 128, N_HI, D)
    # b1 pre-transposed to [128, 32] ([p, hk] = b1[hk*128+p]); b2
    # pre-broadcast across partitions to [128, D].
    b1q = np.ascontiguousarray(
        np.asarray(fc1_b, dtype=np.float32)
        .reshape(E, H // 128, 128).transpose(0, 2, 1))
    b2q = np.ascontiguousarray(np.broadcast_to(
        np.asarray(fc2_b, dtype=np.float32).reshape(E, 1, D), (E, 128, D)))

    in_maps = [
        {
            "xt": xq[e],
            "fc1_w": w1q[e],
            "fc1_b": b1q[e],
            "fc2_w": w2q[e],
            "fc2_b": b2q[e],
        }
        for e in range(E)
    ]
    kw = {}
    if _trace:
        kw = dict(trace=True,
                  trace_cores=_trace_cores if _trace_cores is not None else [0])
    res = run_bass_kernel_spmd(nc, in_maps, core_ids=list(range(NCORES)), **kw)
    out = np.stack([res.results[e]["out"] for e in range(E)], axis=0)
    if _trace:
        return out, res
    return out


# revision 36
# speedup vs baseline: 1.0152x; 1.0152x over previous
"""Expert-parallel batched-expert FFN kernel for Trainium2 (8 NeuronCores).

Reference computation (per expert e):
    y = relu(x[e] @ fc1_w[e] + fc1_b[e]) @ fc2_w[e] + fc2_b[e]

Sharding: E=8 experts, one expert per core (expert parallel, no collectives).

Per-core algorithm (T=2048 tokens, D=1024, H=4096), token-group-outer:
  - All large operands are repacked host-side (with the fp32->fp16 cast)
    into the exact SBUF slab layouts the kernel consumes, so every device
    DMA is one ~1MB transfer with >=1KB-contiguous per-partition rows:
      xt  [c4*128+p][k][t]  (x transposed to [D,T], chunked by 512 tokens)
      w1  [b*128+p][k][h]   (FC1 lhsT tiles, per 512-wide h block)
      w2  [b*128+p][hk][d]  (FC2 rhs tiles, per block)
  - Both weight matrices are SBUF-resident (16MB fp16 = 128KB/partition);
    they stream in once behind the ramp-critical slabs (x c0 + w1(0)).
  - Outer loop over 4 groups of 512 tokens.  Per group:
      FC1: for each of 32 h-tiles, accumulate 8 k-tile matmuls in PSUM,
           relu (+b1, fused) drains to a [128, 512] yT tile (scalar eng).
      FC2: for each (ti, dc) output tile, accumulate ALL 32 h-tiles in a
           single PSUM pass, then one DVE add (+b2 broadcast) drains to
           SBUF and the 256KB store issues immediately.
    So there is no cross-block SBUF accumulator, x chunks 1-3 stay out of
    the DMA ramp, and output stores spread across the whole run instead
    of flushing 8MB at the end.
  - x chunks use a 2-slot SBUF window; the slot-reuse WAR dependency
    auto-delays chunk c+2's DMA until group c's FC1 finished.
  - Matmul operands are fp16 (m10): inputs round to ~2^-11 relative; all
    accumulation is fp32 in PSUM.  Measured end-to-end L2 relative error
    vs the fp32 reference is ~4e-4.
  - Dependency-free REAL matmuls (not transposes, which don't count as
    PE-busy for the HAM clock gate) at t=0 bring the PE clock to 8/8
    during the DMA-bound lead-in so FC1 starts at full rate.
"""

from contextlib import ExitStack

import numpy as np

import concourse.bass as bass
import concourse.bacc as bacc
import concourse.mybir as mybir
import concourse.tile as tile
from concourse.bass_utils import run_bass_kernel_spmd

E, T, D, H = 8, 2048, 1024, 4096
NCORES = 8
HB = 512           # h per weight block
FP = mybir.dt.float32
FP16 = mybir.dt.float16
RELU = mybir.ActivationFunctionType.Relu

N_BLK = H // HB                # 8   weight blocks
N_HI = HB // 128               # 4   h-tiles per block
N_HK = H // 128                # 32  h-tiles total
N_KI = D // 128                # 8   k-tiles for FC1
N_DC = D // 512                # 2   512-col chunks of D
N_C4 = T // 512                # 4   512-token groups
N_TG = 4                       # ti tiles per token group
N_JUNK = 60                    # HAM warm-up matmuls at t=0


def _emit_kernel(tc, out, xt, w1, b1, w2, b2):
    nc = tc.nc
    with ExitStack() as ctx:
        singles = ctx.enter_context(tc.tile_pool(name="singles", bufs=1))
        xt_pool = ctx.enter_context(tc.tile_pool(name="xt", bufs=1))
        yt_pool = ctx.enter_context(tc.tile_pool(name="yt", bufs=1))
        st_pool = ctx.enter_context(tc.tile_pool(name="st", bufs=4))
        w1_pool = ctx.enter_context(tc.tile_pool(name="w1", bufs=1))
        w2_pool = ctx.enter_context(tc.tile_pool(name="w2", bufs=1))
        psum = ctx.enter_context(tc.tile_pool(name="psum", bufs=4, space="PSUM"))

        # ---- ramp-critical DMA order: x c0 + w1 blocks interleaved over
        # BOTH rings in first-use order; the w2 stream (not needed until
        # FC2(c0) at ~70us) is gated behind a junk-matmul landmark via a
        # dummy-tile WAR dependency, so the first ~13us of chip-wide HBM
        # bandwidth carries only ramp-critical bytes. ----
        # sync ring:   x c0 | w1(4..7) | [gate] w2(4..7) | b2b | x c1
        # scalar ring: b1t | w1(0..3)  | [gate] w2(0..3) | x c2 | x c3
        xTc = [None] * N_C4

        def emit_xload(c4, eng):
            xTc[c4] = xt_pool.tile([128, N_KI, 512], FP16, tag=f"xt{c4 % 2}",
                                   name=f"xT{c4}")
            eng.dma_start(out=xTc[c4], in_=xt[c4 * 128:(c4 + 1) * 128, :, :])

        emit_xload(0, nc.sync)

        b1t = singles.tile([128, N_HK], FP)
        nc.scalar.dma_start(out=b1t, in_=b1)

        w1b = []
        for b in range(N_BLK):
            wb = w1_pool.tile([128, N_KI, HB], FP16, tag=f"w1b{b}",
                              name=f"w1b{b}")
            eng = nc.scalar if b < 4 else nc.sync
            eng.dma_start(out=wb, in_=w1[b * 128:(b + 1) * 128, :, :])
            w1b.append(wb)

        wtile = singles.tile([128, 128], FP16)
        nc.vector.memset(wtile, 0.0)

        # dummy first-generation tiles in every w2 ring slot; junk matmuls
        # near the end of the warm-up train consume them, so the real w2
        # loads (slot reuse = WAR) can't start before ~junk-end.
        w2gate = []
        for b in range(N_BLK):
            dm = w2_pool.tile([128, N_HI, D], FP16, tag=f"w2b{b}",
                              name=f"w2dm{b}")
            nc.vector.memset(dm[:, 0, 0:128], 0.0)
            w2gate.append(dm)

        # ---- HAM warm-up: dependency-free real matmuls on a zero tile
        # bring the PE clock gate to 8/8 during the DMA-bound lead-in.
        for j in range(N_JUNK):
            pt = psum.tile([128, 128], FP, tag="psA", name=f"wu{j}")
            nc.tensor.matmul(pt, lhsT=wtile, rhs=wtile, start=True, stop=True)

        emit_xload(1, nc.sync)

        w2b = []
        for b in range(N_BLK):
            wb = w2_pool.tile([128, N_HI, D], FP16, tag=f"w2b{b}",
                              name=f"w2b{b}")
            eng = nc.scalar if b < 4 else nc.sync
            eng.dma_start(out=wb, in_=w2[b * 128:(b + 1) * 128, :, :])
            w2b.append(wb)

        b2b = singles.tile([128, D], FP)
        nc.sync.dma_start(out=b2b, in_=b2)

        emit_xload(2, nc.scalar)   # WAR on slot 0 delays this past FC1(c0)
        emit_xload(3, nc.scalar)   # WAR on slot 1 delays this past FC1(c1)

        yT = [yt_pool.tile([128, 512], FP16, tag=f"yt{hk}", name=f"yT{hk}")
              for hk in range(N_HK)]

        for c4 in range(N_C4):
            # ---- FC1: yT[hk] = relu(w1.T @ x[c4-chunk] + b1) ----
            for b in range(N_BLK):
                pts = [psum.tile([128, 512], FP, tag="psA",
                                 name=f"psfc1_{c4}_{b}_{hi}")
                       for hi in range(N_HI)]
                for hi in range(N_HI):
                    hk = b * N_HI + hi
                    for ki in range(N_KI):
                        nc.tensor.matmul(
                            pts[hi],
                            lhsT=w1b[b][:, ki, hi * 128:(hi + 1) * 128],
                            rhs=xTc[c4][:, ki, :],
                            start=(ki == 0), stop=(ki == N_KI - 1))
                    nc.scalar.activation(
                        out=yT[hk], in_=pts[hi],
                        func=RELU, bias=b1t[:, hk:hk + 1], scale=1.0)
                if c4 == 0 and b == 2:
                    # release the w2 stream here (~35us): junk matmuls
                    # consume the ring-slot dummies, so the w2 DMAs'
                    # WAR waits clear only once FC1 is well underway
                    # and the w1 stream has drained.
                    for g in range(N_BLK):
                        pt = psum.tile([128, 128], FP, tag="psA",
                                       name=f"wug{g}")
                        nc.tensor.matmul(pt, lhsT=wtile,
                                         rhs=w2gate[g][:, 0, 0:128],
                                         start=True, stop=True)

            # ---- FC2: one full-H PSUM pass per (ti, dc) output tile ----
            for ti in range(N_TG):
                gti = c4 * N_TG + ti
                for dc in range(N_DC):
                    pt = psum.tile([128, 512], FP, tag="psB",
                                   name=f"psfc2_{c4}_{ti}_{dc}")
                    for hk in range(N_HK):
                        nc.tensor.matmul(
                            pt,
                            lhsT=yT[hk][:, ti * 128:(ti + 1) * 128],
                            rhs=w2b[hk // N_HI][:, hk % N_HI,
                                                dc * 512:(dc + 1) * 512],
                            start=(hk == 0), stop=(hk == N_HK - 1))
                    st = st_pool.tile([128, 512], FP, tag="st",
                                      name=f"st{gti}_{dc}")
                    nc.vector.tensor_add(
                        st, pt, b2b[:, dc * 512:(dc + 1) * 512])
                    eng = nc.sync if (gti + dc) % 2 == 0 else nc.scalar
                    eng.dma_start(
                        out=out[gti * 128:(gti + 1) * 128,
                                dc * 512:(dc + 1) * 512],
                        in_=st)


def build_module():
    nc = bacc.Bacc("TRN2", target_bir_lowering=False, debug=False)
    xt = nc.dram_tensor("xt", [N_C4 * 128, N_KI, 512], FP16,
                        kind="ExternalInput").ap()
    w1 = nc.dram_tensor("fc1_w", [N_BLK * 128, N_KI, HB], FP16,
                        kind="ExternalInput").ap()
    b1 = nc.dram_tensor("fc1_b", [128, H // 128], FP,
                        kind="ExternalInput").ap()
    w2 = nc.dram_tensor("fc2_w", [N_BLK * 128, N_HI, D], FP16,
                        kind="ExternalInput").ap()
    b2 = nc.dram_tensor("fc2_b", [128, D], FP, kind="ExternalInput").ap()
    out = nc.dram_tensor("out", [T, D], FP, kind="ExternalOutput").ap()
    with tile.TileContext(nc) as tc:
        _emit_kernel(tc, out, xt, w1, b1, w2, b2)
    nc.compile()
    return nc


_CACHED = None


def kernel(x, fc1_w, fc1_b, fc2_w, fc2_b, _trace=False, _trace_cores=None):
    global _CACHED
    if _CACHED is None:
        _CACHED = build_module()
    nc = _CACHED

    # host-side staging: fp16 cast + repack into the kernel's slab layouts
    x16 = np.asarray(x, dtype=np.float32).astype(np.float16)
    w116 = np.asarray(fc1_w, dtype=np.float32).astype(np.float16)
    w216 = np.asarray(fc2_w, dtype=np.float32).astype(np.float16)
    # x [E,T,D] -> xT [E,D,T] -> [E, k, p, c4, t] -> [E, c4, p, k, t]
    xq = np.ascontiguousarray(
        x16.transpose(0, 2, 1).reshape(E, N_KI, 128, N_C4, 512)
           .transpose(0, 3, 2, 1, 4)).reshape(E, N_C4 * 128, N_KI, 512)
    # w1 [E,D,H] -> [E, k, p, b, h] -> [E, b, p, k, h]
    w1q = np.ascontiguousarray(
        w116.reshape(E, N_KI, 128, N_BLK, HB).transpose(0, 3, 2, 1, 4)
    ).reshape(E, N_BLK * 128, N_KI, HB)
    # w2 [E,H,D] -> [E, b, hk, p, d] -> [E, b, p, hk, d]
    w2q = np.ascontiguousarray(
        w216.reshape(E, N_BLK, N_HI, 128, D).transpose(0, 1, 3, 2, 4)
    ).reshape(E, N_BLK * 128, N_HI, D)
    # b1 pre-transposed to [128, 32] ([p, hk] = b1[hk*128+p]); b2
    # pre-broadcast across partitions to [128, D].
    b1q = np.ascontiguousarray(
        np.asarray(fc1_b, dtype=np.float32)
        .reshape(E, H // 128, 128).transpose(0, 2, 1))
    b2q = np.ascontiguousarray(np.broadcast_to(
        np.asarray(fc2_b, dtype=np.float32).reshape(E, 1, D), (E, 128, D)))

    in_maps = [
        {
            "xt": xq[e],
            "fc1_w": w1q[e],
            "fc1_b": b1q[e],
            "fc2_w": w2q[e],
            "fc2_b": b2q[e],
        }
        for e in range(E)
    ]
    kw = {}
    if _trace:
        kw = dict(trace=True,
                  trace_cores=_trace_cores if _trace_cores is not None else [0])
    res = run_bass_kernel_spmd(nc, in_maps, core_ids=list(range(NCORES)), **kw)
    out = np.stack([res.results[e]["out"] for e in range(E)], axis=0)
    if _trace:
        return out, res
    return out


# revision 38
# speedup vs baseline: 1.0348x; 1.0194x over previous
"""Expert-parallel batched-expert FFN kernel for Trainium2 (8 NeuronCores).

Reference computation (per expert e):
    y = relu(x[e] @ fc1_w[e] + fc1_b[e]) @ fc2_w[e] + fc2_b[e]

Sharding: E=8 experts, one expert per core (expert parallel, no collectives).

Per-core algorithm (T=2048 tokens, D=1024, H=4096):
  - All large operands are repacked host-side (with the fp32->fp16 cast)
    into the exact SBUF slab layouts the kernel consumes, so every device
    DMA is one ~1MB transfer with 8KB-contiguous per-partition rows:
      xt  [c4*128+p][k][t]  (x transposed to [D,T], chunked by 512 tokens)
      w1  [b*128+p][k][h]   (FC1 lhsT tiles, per 512-wide h block)
      w2  [b*128+p][hk][d]  (FC2 rhs tiles, per block)
    This keeps the DMA rings at full rate during the rampup, where all 8
    cores contend for HBM (~2us fixed cost per dma_start, ~860KB knee).
  - FC1 produces yT [H, T] so FC2 can consume it as the stationary operand
    directly; w1/w2 stream once, on the scalar ring behind x chunks 1,3;
    x chunks 0,2 ride the sync ring (per-ring bandwidth is the ramp
    bottleneck, so the critical 2MB is split across both rings).
  - FC1 is grouped per (block, 512-token chunk): 4 h-tiles accumulate
    their 8 k-tile matmuls in 4 PSUM banks, relu (+b1, fused) drains each
    on the scalar engine.  FC2 accumulates each block's 4 k-tiles in
    PSUM, then a DVE add folds the partial into the fp32 SBUF
    accumulator (bias b2 folded into the first add); accumulators pair
    two token tiles so the final stores are 8 x 1MB.
  - Matmul operands are fp16 (m10): inputs round to ~2^-11 relative; all
    accumulation is fp32 in PSUM / SBUF.  Measured end-to-end L2 relative
    error vs the fp32 reference is ~4e-4.
  - Dependency-free REAL matmuls (not transposes, which don't count as
    PE-busy for the HAM clock gate) at t=0 bring the PE clock to 8/8
    during the DMA-bound lead-in so FC1 starts at full rate.
"""

from contextlib import ExitStack

import numpy as np

import concourse.bass as bass
import concourse.bacc as bacc
import concourse.mybir as mybir
import concourse.tile as tile
from concourse.bass_utils import run_bass_kernel_spmd

E, T, D, H = 8, 2048, 1024, 4096
NCORES = 8
HB = 512           # h per stream block
FP = mybir.dt.float32
FP16 = mybir.dt.float16
RELU = mybir.ActivationFunctionType.Relu

N_BLK = H // HB                # 8
N_HI = HB // 128               # 4  h-tiles per block
N_KI = D // 128                # 8  k-tiles for FC1
N_TI = T // 128                # 16 token tiles
N_DC = D // 512                # 2  512-col chunks of D
N_C4 = T // 512                # 4  512-token chunks
N_JUNK = 100                   # HAM warm-up matmuls at t=0


def _emit_kernel(tc, out, xt, w1, b1, w2, b2):
    nc = tc.nc
    with ExitStack() as ctx:
        singles = ctx.enter_context(tc.tile_pool(name="singles", bufs=1))
        xt_pool = ctx.enter_context(tc.tile_pool(name="xt", bufs=1))
        yt_pool = ctx.enter_context(tc.tile_pool(name="yt", bufs=N_HI))
        acc_pool = ctx.enter_context(tc.tile_pool(name="acc", bufs=1))
        w1_pool = ctx.enter_context(tc.tile_pool(name="w1", bufs=2))
        w2_pool = ctx.enter_context(tc.tile_pool(name="w2", bufs=2))
        psum = ctx.enter_context(tc.tile_pool(name="psum", bufs=4, space="PSUM"))

        # xTc[c4] [128, 8, 512]: [d%128, d//128, t] slab per token chunk.
        xTc = [xt_pool.tile([128, N_KI, 512], FP16, tag=f"xt{c4}",
                            name=f"xT{c4}") for c4 in range(N_C4)]

        def emit_xload(c4, eng):
            eng.dma_start(out=xTc[c4],
                          in_=xt[c4 * 128:(c4 + 1) * 128, :, :])

        def emit_w1(b):
            wb = w1_pool.tile([128, N_KI, HB], FP16, tag="w1", name=f"w1b{b}")
            nc.scalar.dma_start(out=wb, in_=w1[b * 128:(b + 1) * 128, :, :])
            return wb

        def emit_w2(b):
            wb = w2_pool.tile([128, N_HI, D], FP16, tag="w2", name=f"w2b{b}")
            nc.scalar.dma_start(out=wb, in_=w2[b * 128:(b + 1) * 128, :, :])
            return wb

        # ---- ramp-critical DMA order (biases are pre-transposed /
        # pre-broadcast host-side so every transfer is a clean slab).
        # One slab per ring on the critical path (x c0 on sync, w1(0) on
        # scalar); the rest queues behind, ordered by first-use time. ----
        # sync ring:   x c0 | w2(0) | b2b   (+ half the stores later)
        # scalar ring: w1(0) | b1t | x c1 | x c2 | x c3 | w1(1) | w2(1)...
        emit_xload(0, nc.sync)
        w1p_cur = emit_w1(0)

        b1t = singles.tile([128, H // 128], FP)
        nc.scalar.dma_start(out=b1t, in_=b1)

        emit_xload(1, nc.scalar)
        emit_xload(2, nc.scalar)
        emit_xload(3, nc.scalar)

        wtile = singles.tile([128, 128], FP16)
        nc.vector.memset(wtile, 0.0)

        w2t_cur = w2_pool.tile([128, N_HI, D], FP16, tag="w2", name="w2b0")
        nc.sync.dma_start(out=w2t_cur, in_=w2[0:128, :, :])

        b2b = singles.tile([128, D], FP)
        nc.sync.dma_start(out=b2b, in_=b2)

        # ---- HAM warm-up: dependency-free real matmuls on a zero tile
        # bring the PE clock gate to 8/8 during the DMA-bound lead-in.
        for j in range(N_JUNK):
            pt = psum.tile([128, 128], FP, tag="psA", name=f"wu{j}")
            nc.tensor.matmul(pt, lhsT=wtile, rhs=wtile, start=True, stop=True)

        # paired-token-tile accumulators: 8 x [128, 2, 1024] fp32
        accs = [acc_pool.tile([128, 2, D], FP, tag=f"acc{g}", name=f"acc{g}")
                for g in range(N_TI // 2)]

        yTb = None
        for b in range(N_BLK):
            w1p = w1p_cur if b == 0 else emit_w1(b)
            w2t = w2t_cur if b == 0 else emit_w2(b)

            # ---- FC1: yT block [HB, T] = relu(w1.T @ xT + b1), grouped
            # per 512-token chunk so block 0 starts on chunk 0 alone ----
            yTb = [yt_pool.tile([128, T], FP16, tag="yt",
                                name=f"yT{b}_{i}")
                   for i in range(N_HI)]
            for c4 in range(N_C4):
                pts = [psum.tile([128, 512], FP, tag="psA",
                                 name=f"psfc1_{b}_{c4}_{hi}")
                       for hi in range(N_HI)]
                for hi in range(N_HI):
                    h_abs = b * N_HI + hi
                    for ki in range(N_KI):
                        nc.tensor.matmul(
                            pts[hi],
                            lhsT=w1p[:, ki, hi * 128:(hi + 1) * 128],
                            rhs=xTc[c4][:, ki, :],
                            start=(ki == 0), stop=(ki == N_KI - 1))
                    nc.scalar.activation(
                        out=yTb[hi][:, c4 * 512:(c4 + 1) * 512],
                        in_=pts[hi],
                        func=RELU, bias=b1t[:, h_abs:h_abs + 1], scale=1.0)

            # ---- FC2 partial: acc += yTb.T @ w2[block] ----
            for ti in range(N_TI):
                pts = [psum.tile([128, 512], FP, tag="psB",
                                 name=f"psfc2_{b}_{ti}_{d}")
                       for d in range(N_DC)]
                for hk in range(N_HI):
                    for dc in range(N_DC):
                        nc.tensor.matmul(
                            pts[dc],
                            lhsT=yTb[hk][:, ti * 128:(ti + 1) * 128],
                            rhs=w2t[:, hk, dc * 512:(dc + 1) * 512],
                            start=(hk == 0), stop=(hk == N_HI - 1))
                for dc in range(N_DC):
                    asl = accs[ti // 2][:, ti % 2, dc * 512:(dc + 1) * 512]
                    if b == 0:
                        nc.vector.tensor_add(
                            asl, pts[dc],
                            b2b[:, dc * 512:(dc + 1) * 512])
                    else:
                        nc.vector.tensor_add(asl, asl, pts[dc])

        # ---- store: 16 x 512KB alternating across both rings ----
        outr = out.rearrange("(r p) d -> p r d", p=128)
        for ti in range(N_TI):
            eng = nc.sync if ti % 2 == 0 else nc.scalar
            eng.dma_start(out=outr[:, ti, :],
                          in_=accs[ti // 2][:, ti % 2, :])


def build_module():
    nc = bacc.Bacc("TRN2", target_bir_lowering=False, debug=False)
    xt = nc.dram_tensor("xt", [N_C4 * 128, N_KI, 512], FP16,
                        kind="ExternalInput").ap()
    w1 = nc.dram_tensor("fc1_w", [N_BLK * 128, N_KI, HB], FP16,
                        kind="ExternalInput").ap()
    b1 = nc.dram_tensor("fc1_b", [128, H // 128], FP,
                        kind="ExternalInput").ap()
    w2 = nc.dram_tensor("fc2_w", [N_BLK * 128, N_HI, D], FP16,
                        kind="ExternalInput").ap()
    b2 = nc.dram_tensor("fc2_b", [128, D], FP, kind="ExternalInput").ap()
    out = nc.dram_tensor("out", [T, D], FP, kind="ExternalOutput").ap()
    with tile.TileContext(nc) as tc:
        _emit_kernel(tc, out, xt, w1, b1, w2, b2)
    nc.compile()
    return nc


_CACHED = None


def kernel(x, fc1_w, fc1_b, fc2_w, fc2_b, _trace=False, _trace_cores=None):
    global _CACHED
    if _CACHED is None:
        _CACHED = build_module()
    nc = _CACHED

    # host-side staging: fp16 cast + repack into the kernel's slab layouts
    x16 = np.asarray(x, dtype=np.float32).astype(np.float16)
    w116 = np.asarray(fc1_w, dtype=np.float32).astype(np.float16)
    w216 = np.asarray(fc2_w, dtype=np.float32).astype(np.float16)
    # x [E,T,D] -> xT [E,D,T] -> [E, k, p, c4, t] -> [E, c4, p, k, t]
    xq = np.ascontiguousarray(
        x16.transpose(0, 2, 1).reshape(E, N_KI, 128, N_C4, 512)
           .transpose(0, 3, 2, 1, 4)).reshape(E, N_C4 * 128, N_KI, 512)
    # w1 [E,D,H] -> [E, k, p, b, h] -> [E, b, p, k, h]
    w1q = np.ascontiguousarray(
        w116.reshape(E, N_KI, 128, N_BLK, HB).transpose(0, 3, 2, 1, 4)
    ).reshape(E, N_BLK * 128, N_KI, HB)
    # w2 [E,H,D] -> [E, b, hk, p, d] -> [E, b, p, hk, d]
    w2q = np.ascontiguousarray(
        w216.reshape(E, N_BLK, N_HI, 128, D).transpose(0, 1, 3, 2, 4)
    ).reshape(E, N_BLK * 128, N_HI, D)
    # b1 pre-transposed to [128, 32] ([p, hi] = b1[hi*128+p]); b2
    # pre-broadcast across partitions to [128, D].
    b1q = np.ascontiguousarray(
        np.asarray(fc1_b, dtype=np.float32)
        .reshape(E, H // 128, 128).transpose(0, 2, 1))
    b2q = np.ascontiguousarray(np.broadcast_to(
        np.asarray(fc2_b, dtype=np.float32).reshape(E, 1, D), (E, 128, D)))

    in_maps = [
        {
            "xt": xq[e],
            "fc1_w": w1q[e],
            "fc1_b": b1q[e],
            "fc2_w": w2q[e],
            "fc2_b": b2q[e],
        }
        for e in range(E)
    ]
    kw = {}
    if _trace:
        kw = dict(trace=True,
                  trace_cores=_trace_cores if _trace_cores is not None else [0])
    res = run_bass_kernel_spmd(nc, in_maps, core_ids=list(range(NCORES)), **kw)
    out = np.stack([res.results[e]["out"] for e in range(E)], axis=0)
    if _trace:
        return out, res
    return out


# revision 39
# speedup vs baseline: 1.0368x; 1.0019x over previous
"""Expert-parallel batched-expert FFN kernel for Trainium2 (8 NeuronCores).

Reference computation (per expert e):
    y = relu(x[e] @ fc1_w[e] + fc1_b[e]) @ fc2_w[e] + fc2_b[e]

Sharding: E=8 experts, one expert per core (expert parallel, no collectives).

Per-core algorithm (T=2048 tokens, D=1024, H=4096):
  - All large operands are repacked host-side (with the fp32->fp16 cast)
    into the exact SBUF slab layouts the kernel consumes, so every device
    DMA is one ~1MB transfer with 8KB-contiguous per-partition rows:
      xt  [c4*128+p][k][t]  (x transposed to [D,T], chunked by 512 tokens)
      w1  [b*128+p][k][h]   (FC1 lhsT tiles, per 512-wide h block)
      w2  [b*128+p][hk][d]  (FC2 rhs tiles, per block)
    This keeps the DMA rings at full rate during the rampup, where all 8
    cores contend for HBM (~2us fixed cost per dma_start, ~860KB knee).
  - FC1 produces yT [H, T] so FC2 can consume it as the stationary operand
    directly; weights stream once.  Ramp-critical slabs get one ring
    each (x c0 on sync, w1(0) on scalar; per-ring bandwidth is the ramp
    bottleneck); x c1-c3 + later weight blocks queue on the scalar ring,
    w2(0)/b2b on the sync ring, ordered by first-use time.
  - FC1 is grouped per (block, 512-token chunk): 4 h-tiles accumulate
    their 8 k-tile matmuls in 4 PSUM banks, relu (+b1, fused) drains each
    on the scalar engine.  FC2 accumulates each block's 4 k-tiles in
    PSUM, then a DVE add folds the partial into the fp32 SBUF
    accumulator (bias b2 folded into the first add); the 16 output
    stores alternate across both rings so the final flush is short.
  - Matmul operands are fp16 (m10): inputs round to ~2^-11 relative; all
    accumulation is fp32 in PSUM / SBUF.  Measured end-to-end L2 relative
    error vs the fp32 reference is ~4e-4.
  - Dependency-free REAL matmuls (not transposes, which don't count as
    PE-busy for the HAM clock gate) at t=0 bring the PE clock to 8/8
    during the DMA-bound lead-in so FC1 starts at full rate.
"""

from contextlib import ExitStack

import numpy as np

import concourse.bass as bass
import concourse.bacc as bacc
import concourse.mybir as mybir
import concourse.tile as tile
from concourse.bass_utils import run_bass_kernel_spmd

E, T, D, H = 8, 2048, 1024, 4096
NCORES = 8
HB = 512           # h per stream block
FP = mybir.dt.float32
FP16 = mybir.dt.float16
RELU = mybir.ActivationFunctionType.Relu

N_BLK = H // HB                # 8
N_HI = HB // 128               # 4  h-tiles per block
N_KI = D // 128                # 8  k-tiles for FC1
N_TI = T // 128                # 16 token tiles
N_DC = D // 512                # 2  512-col chunks of D
N_C4 = T // 512                # 4  512-token chunks
N_JUNK = 100                   # HAM warm-up matmuls at t=0


def _emit_kernel(tc, out, xt, w1, b1, w2, b2):
    nc = tc.nc
    with ExitStack() as ctx:
        singles = ctx.enter_context(tc.tile_pool(name="singles", bufs=1))
        xt_pool = ctx.enter_context(tc.tile_pool(name="xt", bufs=1))
        yt_pool = ctx.enter_context(tc.tile_pool(name="yt", bufs=N_HI))
        acc_pool = ctx.enter_context(tc.tile_pool(name="acc", bufs=1))
        w1_pool = ctx.enter_context(tc.tile_pool(name="w1", bufs=2))
        w2_pool = ctx.enter_context(tc.tile_pool(name="w2", bufs=2))
        psum = ctx.enter_context(tc.tile_pool(name="psum", bufs=4, space="PSUM"))

        # xTc[c4] [128, 8, 512]: [d%128, d//128, t] slab per token chunk.
        xTc = [xt_pool.tile([128, N_KI, 512], FP16, tag=f"xt{c4}",
                            name=f"xT{c4}") for c4 in range(N_C4)]

        def emit_xload(c4, eng):
            eng.dma_start(out=xTc[c4],
                          in_=xt[c4 * 128:(c4 + 1) * 128, :, :])

        def emit_w1(b):
            wb = w1_pool.tile([128, N_KI, HB], FP16, tag="w1", name=f"w1b{b}")
            nc.scalar.dma_start(out=wb, in_=w1[b * 128:(b + 1) * 128, :, :])
            return wb

        def emit_w2(b):
            wb = w2_pool.tile([128, N_HI, D], FP16, tag="w2", name=f"w2b{b}")
            nc.scalar.dma_start(out=wb, in_=w2[b * 128:(b + 1) * 128, :, :])
            return wb

        # ---- ramp-critical DMA order (biases are pre-transposed /
        # pre-broadcast host-side so every transfer is a clean slab).
        # One slab per ring on the critical path (x c0 on sync, w1(0) on
        # scalar); the rest queues behind, ordered by first-use time. ----
        # sync ring:   x c0 | w2(0) | b2b   (+ half the stores later)
        # scalar ring: w1(0) | b1t | x c1 | x c2 | x c3 | w1(1) | w2(1)...
        emit_xload(0, nc.sync)
        w1p_cur = emit_w1(0)

        b1t = singles.tile([128, H // 128], FP)
        nc.scalar.dma_start(out=b1t, in_=b1)

        emit_xload(1, nc.scalar)
        emit_xload(2, nc.scalar)
        emit_xload(3, nc.scalar)

        wtile = singles.tile([128, 128], FP16)
        nc.vector.memset(wtile, 0.0)

        w2t_cur = w2_pool.tile([128, N_HI, D], FP16, tag="w2", name="w2b0")
        nc.sync.dma_start(out=w2t_cur, in_=w2[0:128, :, :])

        b2b = singles.tile([128, D], FP)
        nc.sync.dma_start(out=b2b, in_=b2)

        # ---- HAM warm-up: dependency-free real matmuls on a zero tile
        # bring the PE clock gate to 8/8 during the DMA-bound lead-in.
        for j in range(N_JUNK):
            pt = psum.tile([128, 128], FP, tag="psA", name=f"wu{j}")
            nc.tensor.matmul(pt, lhsT=wtile, rhs=wtile, start=True, stop=True)

        # paired-token-tile accumulators: 8 x [128, 2, 1024] fp32
        accs = [acc_pool.tile([128, 2, D], FP, tag=f"acc{g}", name=f"acc{g}")
                for g in range(N_TI // 2)]

        yTb = None
        for b in range(N_BLK):
            w1p = w1p_cur if b == 0 else emit_w1(b)
            w2t = w2t_cur if b == 0 else emit_w2(b)

            # ---- FC1: yT block [HB, T] = relu(w1.T @ xT + b1), grouped
            # per 512-token chunk so block 0 starts on chunk 0 alone ----
            yTb = [yt_pool.tile([128, T], FP16, tag="yt",
                                name=f"yT{b}_{i}")
                   for i in range(N_HI)]
            for c4 in range(N_C4):
                pts = [psum.tile([128, 512], FP, tag="psA",
                                 name=f"psfc1_{b}_{c4}_{hi}")
                       for hi in range(N_HI)]
                for hi in range(N_HI):
                    h_abs = b * N_HI + hi
                    for ki in range(N_KI):
                        nc.tensor.matmul(
                            pts[hi],
                            lhsT=w1p[:, ki, hi * 128:(hi + 1) * 128],
                            rhs=xTc[c4][:, ki, :],
                            start=(ki == 0), stop=(ki == N_KI - 1))
                    nc.scalar.activation(
                        out=yTb[hi][:, c4 * 512:(c4 + 1) * 512],
                        in_=pts[hi],
                        func=RELU, bias=b1t[:, h_abs:h_abs + 1], scale=1.0)

            # ---- FC2 partial: acc += yTb.T @ w2[block] ----
            for ti in range(N_TI):
                pts = [psum.tile([128, 512], FP, tag="psB",
                                 name=f"psfc2_{b}_{ti}_{d}")
                       for d in range(N_DC)]
                for hk in range(N_HI):
                    for dc in range(N_DC):
                        nc.tensor.matmul(
                            pts[dc],
                            lhsT=yTb[hk][:, ti * 128:(ti + 1) * 128],
                            rhs=w2t[:, hk, dc * 512:(dc + 1) * 512],
                            start=(hk == 0), stop=(hk == N_HI - 1))
                for dc in range(N_DC):
                    asl = accs[ti // 2][:, ti % 2, dc * 512:(dc + 1) * 512]
                    if b == 0:
                        nc.vector.tensor_add(
                            asl, pts[dc],
                            b2b[:, dc * 512:(dc + 1) * 512])
                    else:
                        nc.vector.tensor_add(asl, asl, pts[dc])

        # ---- store: 16 x 512KB alternating across both rings ----
        outr = out.rearrange("(r p) d -> p r d", p=128)
        for ti in range(N_TI):
            eng = nc.sync if ti % 2 == 0 else nc.scalar
            eng.dma_start(out=outr[:, ti, :],
                          in_=accs[ti // 2][:, ti % 2, :])


def build_module():
    nc = bacc.Bacc("TRN2", target_bir_lowering=False, debug=False)
    xt = nc.dram_tensor("xt", [N_C4 * 128, N_KI, 512], FP16,
                        kind="ExternalInput").ap()
    w1 = nc.dram_tensor("fc1_w", [N_BLK * 128, N_KI, HB], FP16,
                        kind="ExternalInput").ap()
    b1 = nc.dram_tensor("fc1_b", [128, H // 128], FP,
                        kind="ExternalInput").ap()
    w2 = nc.dram_tensor("fc2_w", [N_BLK * 128, N_HI, D], FP16,
                        kind="ExternalInput").ap()
    b2 = nc.dram_tensor("fc2_b", [128, D], FP, kind="ExternalInput").ap()
    out = nc.dram_tensor("out", [T, D], FP, kind="ExternalOutput").ap()
    with tile.TileContext(nc) as tc:
        _emit_kernel(tc, out, xt, w1, b1, w2, b2)
    nc.compile()
    return nc


_CACHED = None


def kernel(x, fc1_w, fc1_b, fc2_w, fc2_b, _trace=False, _trace_cores=None):
    global _CACHED
    if _CACHED is None:
        _CACHED = build_module()
    nc = _CACHED

    # host-side staging: fp16 cast + repack into the kernel's slab layouts
    x16 = np.asarray(x, dtype=np.float32).astype(np.float16)
    w116 = np.asarray(fc1_w, dtype=np.float32).astype(np.float16)
    w216 = np.asarray(fc2_w, dtype=np.float32).astype(np.float16)
    # x [E,T,D] -> xT [E,D,T] -> [E, k, p, c4, t] -> [E, c4, p, k, t]
    xq = np.ascontiguousarray(
        x16.transpose(0, 2, 1).reshape(E, N_KI, 128, N_C4, 512)
           .transpose(0, 3, 2, 1, 4)).reshape(E, N_C4 * 128, N_KI, 512)
    # w1 [E,D,H] -> [E, k, p, b, h] -> [E, b, p, k, h]
    w1q = np.ascontiguousarray(
        w116.reshape(E, N_KI, 128, N_BLK, HB).transpose(0, 3, 2, 1, 4)
    ).reshape(E, N_BLK * 128, N_KI, HB)
    # w2 [E,H,D] -> [E, b, hk, p, d] -> [E, b, p, hk, d]
    w2q = np.ascontiguousarray(
        w216.reshape(E, N_BLK, N_HI, 128, D).transpose(0, 1, 3, 2, 4)
    ).reshape(E, N_BLK * 128, N_HI, D)
    # b1 pre-transposed to [128, 32] ([p, hi] = b1[hi*128+p]); b2
    # pre-broadcast across partitions to [128, D].
    b1q = np.ascontiguousarray(
        np.asarray(fc1_b, dtype=np.float32)
        .reshape(E, H // 128, 128).transpose(0, 2, 1))
    b2q = np.ascontiguousarray(np.broadcast_to(
        np.asarray(fc2_b, dtype=np.float32).reshape(E, 1, D), (E, 128, D)))

    in_maps = [
        {
            "xt": xq[e],
            "fc1_w": w1q[e],
            "fc1_b": b1q[e],
            "fc2_w": w2q[e],
            "fc2_b": b2q[e],
        }
        for e in range(E)
    ]
    kw = {}
    if _trace:
        kw = dict(trace=True,
                  trace_cores=_trace_cores if _trace_cores is not None else [0])
    res = run_bass_kernel_spmd(nc, in_maps, core_ids=list(range(NCORES)), **kw)
    out = np.stack([res.results[e]["out"] for e in range(E)], axis=0)
    if _trace:
        return out, res
    return out
